# revision 13
# baseline (speedup 1.0000x reference)
"""AttentiveItemToVec Trainium2 kernel (8 NeuronCores, batch-parallel).

Math (per batch row b):
  v = tvec_w[titems[b]]            [T,E]     (gather)
  u = cvec_w[citems[b]]            [C,E]     (gather)
  t_vec = v @ At_w.T + At_b        [T,DA]
  c_vec = u @ Ac_w.T + Ac_b        [C,DA]
  cos   = (t_vec/|t_vec|) . (c_vec/|c_vec|)   [T,C]
  attn  = softmax(mask(cos))       [T,C]
  z     = (attn @ (u @ Bc_w.T + Bc_b)) @ R_w.T + R_b

Kernel strategy per core (512 batch rows, groups of NB=16 = 128 t-vecs):
  - masked c-items redirected to the PAD row (zeros) host-side, so their u
    is 0; masking becomes a multiplicative 0/1 on attn (on gpsimd), softmax
    sum via ones-matmul after masking.  No -inf biases anywhere.
  - num[t,c] = u_c . g_t + beta_t with g = Ac_w.T @ t_hat, beta = t_hat.Ac_b
    (only T-wide matmuls from the transposed u; bias via K=1 ACC matmul)
  - u gathered C-major bf16 in 2-group batches (2 indirect DMAs / 2 groups
    instead of 2 per batch row: SWDGE fixed cost amortized 32x)
  - uT via PE transposes in [E,2,C] PSUM slabs, evacuated to SBUF bf16
  - cn^2 via cvT = Acw @ uT slabs packed 2x60 partitions, ACT Square(+bias)
    evacuation, per-row ones-matmul column sums, batched Ln/Exp -> invcn
  - invcn applied to PSUM num with one broadcast DVE multiply per chunk,
    then one pure batched Exp per chunk
  - z = ((attn@u)/Sigma @ Bc_w.T) @ R_w.T + (R_w@Bc_b + R_b)
"""

import os
import numpy as np
import ml_dtypes

import concourse.bass as bass
import concourse.bacc as bacc
import concourse.mybir as mybir
import concourse.tile as tile
from concourse.bass_utils import run_bass_kernel_spmd
from concourse.masks import make_identity

F32 = mybir.dt.float32
BF16 = mybir.dt.bfloat16
I32 = mybir.dt.int32
AF = mybir.ActivationFunctionType
OP = mybir.AluOpType

V, E, DA = 100000, 128, 60
B, T, C = 4096, 8, 200
PAD = V - 1
NCORES = 8
BL = B // NCORES          # 512 local batch rows
NB = 16                   # batch rows per group (NB*T = 128 t-vecs)
NG = BL // NB             # 32 groups
C1, C2 = 128, C - 128     # C chunking: 128 + 72
NW = 2 * NB * T + 2 * NB  # ps_big cols: num1 [0:128], num2 [128:256], cn [256:288]

_CACHE: dict = {}


def _pin_act_table():
    """Force every activation onto the natural_log_exp_and_others table.

    All ACT funcs used here (Copy/Identity/Square/Ln/Exp) live in that one
    table; pinning avoids 1.28us ACT_TABLE_LOAD thrash between Exp and Ln.
    """
    from concourse.hw_specs import get_activation_tables
    keep = "natural_log_exp_and_others"
    orig = get_activation_tables("gen3")
    pinned = {k: (v if k == keep else set()) for k, v in orig.items()}
    bacc.get_activation_tables = lambda arch: pinned


def _build():
    _pin_act_table()
    nc = bacc.Bacc(
        "TRN2", target_bir_lowering=False, debug=False, num_devices=NCORES
    )
    d = {}
    def din(name, shape, dt):
        d[name] = nc.dram_tensor(name, list(shape), dt, kind="ExternalInput").ap()
    din("tvec", [V, E], F32)
    din("cvec", [V, E], F32)
    din("acwt", [E, 64], BF16)      # Ac_w.T zero-padded to 64 (cvT lhsT)
    din("acwd", [DA, E], BF16)      # Ac_w             (H matmul lhsT)
    din("acbb", [DA, 1], BF16)      # Ac_b             (beta matmul lhsT)
    din("acb2", [128, 1], F32)      # [Ac_b;0;Ac_b;0]  (Square bias, packed)
    din("atwt", [E, DA], BF16)      # At_w.T
    din("atb", [DA, 1], F32)
    din("bcwt", [E, E], BF16)       # Bc_w.T
    din("rwt", [E, E], BF16)        # R_w.T
    din("rwt32", [E, E], F32)
    din("bcb32", [E, 1], F32)
    din("rb32", [E, 1], F32)
    din("cit1", [C1, BL], I32)      # masked citems (pads -> PAD row)
    din("cit2", [C2, BL], I32)
    din("mb1", [C1, BL], BF16)      # 1.0 valid / 0.0 masked
    din("mb2", [C2, BL], BF16)
    din("titg", [NB * T, NG], I32)
    z_dram = nc.dram_tensor("z_out", [BL * T, E], F32, kind="ExternalOutput").ap()

    with tile.TileContext(nc) as tc:
        with (
            tc.tile_pool(name="const", bufs=1) as cp,
            tc.tile_pool(name="gath", bufs=3) as gp,       # u gathers
            tc.tile_pool(name="tpath", bufs=2) as tp,      # t-path small tiles
            tc.tile_pool(name="usb", bufs=2) as up,        # uT / sq sbuf
            tc.tile_pool(name="att", bufs=2) as ap_,       # attn tiles
            tc.tile_pool(name="zp", bufs=2) as zp,         # z-path tiles
            tc.tile_pool(name="psT", bufs=2, space="PSUM") as psT,   # uT slabs
            tc.tile_pool(name="psC", bufs=2, space="PSUM") as psC,   # cvT slabs
            tc.tile_pool(name="psN", bufs=2, space="PSUM") as psN,   # num+cn
            tc.tile_pool(name="psS", bufs=1, space="PSUM") as psS,   # sT
            tc.tile_pool(name="psG", bufs=1, space="PSUM") as psG,   # t/z path
        ):
            # ---- constants ----
            idb = cp.tile([128, 128], BF16, tag="idb")
            make_identity(nc, idb[:])
            idf = cp.tile([128, 128], F32, tag="idf")
            make_identity(nc, idf[:])
            onesb = cp.tile([128, 1], BF16, tag="onesb")
            nc.gpsimd.memset(onesb[:], 1.0)
            ones_row = cp.tile([1, 128], BF16, tag="onesr")
            nc.gpsimd.memset(ones_row[:], 1.0)
            ones_row32 = cp.tile([1, 128], F32, tag="onesr32")
            nc.gpsimd.memset(ones_row32[:], 1.0)

            acwt = cp.tile([E, 64], BF16, tag="acwt")
            nc.sync.dma_start(acwt[:], d["acwt"][:])
            acwd = cp.tile([DA, E], BF16, tag="acwd")
            nc.sync.dma_start(acwd[:], d["acwd"][:])
            acbb = cp.tile([DA, 1], BF16, tag="acbb")
            nc.sync.dma_start(acbb[:], d["acbb"][:])
            acb2 = cp.tile([128, 1], F32, tag="acb2")
            nc.sync.dma_start(acb2[:], d["acb2"][:])
            atwt = cp.tile([E, DA], BF16, tag="atwt")
            nc.sync.dma_start(atwt[:], d["atwt"][:])
            atb = cp.tile([DA, 1], F32, tag="atb")
            nc.sync.dma_start(atb[:], d["atb"][:])
            bcwt = cp.tile([E, E], BF16, tag="bcwt")
            nc.sync.dma_start(bcwt[:], d["bcwt"][:])
            rwt = cp.tile([E, E], BF16, tag="rwt")
            nc.sync.dma_start(rwt[:], d["rwt"][:])
            rwt32 = cp.tile([E, E], F32, tag="rwt32")
            nc.sync.dma_start(rwt32[:], d["rwt32"][:])
            bcb32 = cp.tile([E, 1], F32, tag="bcb32")
            nc.sync.dma_start(bcb32[:], d["bcb32"][:])
            rb32 = cp.tile([E, 1], F32, tag="rb32")
            nc.sync.dma_start(rb32[:], d["rb32"][:])
            cit1 = cp.tile([C1, BL], I32, tag="cit1")
            nc.sync.dma_start(cit1[:], d["cit1"][:])
            cit2 = cp.tile([C2, BL], I32, tag="cit2")
            nc.sync.dma_start(cit2[:], d["cit2"][:])
            mb1 = cp.tile([C1, BL], BF16, tag="mb1")
            nc.sync.dma_start(mb1[:], d["mb1"][:])
            mb2 = cp.tile([C2, BL], BF16, tag="mb2")
            nc.sync.dma_start(mb2[:], d["mb2"][:])
            titg = cp.tile([NB * T, NG], I32, tag="titg")
            nc.sync.dma_start(titg[:], d["titg"][:])

            # ---- one-time: c2b = broadcast(R_w @ Bc_b + R_b) ----
            ps_c2 = psG.tile([E, 1], F32, space="PSUM", tag="g1")
            nc.tensor.matmul(ps_c2[:], lhsT=rwt32[:], rhs=bcb32[:])
            c2col = cp.tile([E, 1], F32, tag="c2col")
            nc.scalar.activation(c2col[:], ps_c2[:], AF.Identity, bias=rb32[:])
            ps_c2r = psG.tile([1, E], F32, space="PSUM", tag="g1")
            nc.tensor.matmul(ps_c2r[:], lhsT=c2col[:], rhs=idf[:])
            c2row = cp.tile([1, E], F32, tag="c2row")
            nc.scalar.copy(c2row[:], ps_c2r[:])
            ps_c2b = psG.tile([E, E], F32, space="PSUM", tag="g1")
            nc.tensor.matmul(ps_c2b[:], lhsT=ones_row32[:], rhs=c2row[:])
            c2b = cp.tile([E, E], F32, tag="c2b")
            nc.scalar.copy(c2b[:], ps_c2b[:])

            # ---- main loop ----
            for g in range(NG):
                # ---- gathers ([<=128, 1]-offset form; 2 per batch row) ----
                tvg = tp.tile([NB * T, E], BF16, tag="tvg")
                nc.gpsimd.indirect_dma_start(
                    out=tvg[:], out_offset=None, in_=d["tvec"][:],
                    in_offset=bass.IndirectOffsetOnAxis(
                        ap=titg[:, g:g + 1], axis=0),
                )
                ug1 = gp.tile([C1, NB, E], BF16, tag="ug1")
                ug2 = gp.tile([C2, NB, E], BF16, tag="ug2")
                for j in range(NB):
                    b = g * NB + j
                    nc.gpsimd.indirect_dma_start(
                        out=ug1[:, j, :], out_offset=None, in_=d["cvec"][:],
                        in_offset=bass.IndirectOffsetOnAxis(
                            ap=cit1[:, b:b + 1], axis=0),
                    )
                    nc.gpsimd.indirect_dma_start(
                        out=ug2[:, j, :], out_offset=None, in_=d["cvec"][:],
                        in_offset=bass.IndirectOffsetOnAxis(
                            ap=cit2[:, b:b + 1], axis=0),
                    )
                gs = 0

                # ---- t path ----
                ps_vT = psG.tile([E, 128], F32, space="PSUM", tag="g1")
                nc.tensor.matmul(ps_vT[:], lhsT=tvg[:], rhs=idb[:])
                vT = tp.tile([E, 128], BF16, tag="vT")
                nc.vector.tensor_copy(vT[:], ps_vT[:])
                ps_tvT = psG.tile([DA, 128], F32, space="PSUM", tag="g1")
                nc.tensor.matmul(ps_tvT[:], lhsT=atwt[:], rhs=vT[:])
                tvTb = tp.tile([DA, 128], BF16, tag="tvTb")
                nc.scalar.activation(tvTb[:], ps_tvT[:], AF.Identity, bias=atb[:])
                ps_tv = psG.tile([128, DA], F32, space="PSUM", tag="g1")
                nc.tensor.matmul(ps_tv[:], lhsT=tvTb[:], rhs=idb[0:DA, 0:DA])
                tsq = tp.tile([128, DA], BF16, tag="tsq")
                tn2 = tp.tile([128, 1], F32, tag="tn2")
                nc.scalar.activation(tsq[:], ps_tv[:], AF.Square, accum_out=tn2[:])
                ltn = tp.tile([128, 1], F32, tag="ltn")
                nc.scalar.activation(ltn[:], tn2[:], AF.Ln)
                invtn = tp.tile([128, 1], F32, tag="invtn")
                nc.scalar.activation(invtn[:], ltn[:], AF.Exp, scale=-0.5)
                thbt = tp.tile([128, DA], BF16, tag="thbt")
                nc.vector.tensor_scalar_mul(thbt[:], ps_tv[:], invtn[:])
                ps_thT = psG.tile([DA, 128], F32, space="PSUM", tag="g1")
                nc.tensor.matmul(ps_thT[:], lhsT=thbt[:], rhs=idb[:])
                thT = tp.tile([DA, 128], BF16, tag="thT")
                nc.vector.tensor_copy(thT[:], ps_thT[:])
                # H = Ac_w.T @ t_hat  [E, 128];  beta = t_hat . Ac_b  [1, 128]
                ps_HT = psG.tile([E, 128], F32, space="PSUM", tag="g1")
                nc.tensor.matmul(ps_HT[:], lhsT=acwd[:], rhs=thT[:])
                HTsb = tp.tile([E, 128], BF16, tag="HTsb")
                nc.vector.tensor_copy(HTsb[:], ps_HT[:])
                ps_be = psG.tile([1, 128], F32, space="PSUM", tag="g1")
                nc.tensor.matmul(ps_be[:], lhsT=acbb[:], rhs=thT[:])
                berow = tp.tile([1, 128], BF16, tag="berow")
                nc.scalar.copy(berow[:], ps_be[:])

                # ---- u transposes (2-row slabs) + uT evacuation ----
                uTs = up.tile([E, NB, C], BF16, tag="uTs")
                for r in range(NB // 2):
                    ps_uT = psT.tile([E, 2, C], F32, space="PSUM", tag="uT")
                    for h in range(2):
                        j = 2 * r + h
                        nc.tensor.matmul(ps_uT[:, h, 0:C1],
                                         lhsT=ug1[:, gs + j, :], rhs=idb[:])
                        nc.tensor.matmul(ps_uT[:, h, C1:C],
                                         lhsT=ug2[:, gs + j, :],
                                         rhs=idb[0:C2, 0:C2])
                    dst = uTs[:, 2 * r:2 * r + 2, :]
                    if r % 4 < 3:
                        nc.vector.tensor_copy(dst, ps_uT[:])
                    else:
                        nc.scalar.copy(dst, ps_uT[:])

                # ---- cvT slabs (packed 2x64 partitions) + Square -> sq ----
                # sqs[64*(i//2) :, j//4, 200*(i%2) :] holds row j (i = j%4);
                # acwt zero-padded to 64 cols so quadrants are fully written
                sqs = up.tile([128, NB // 4, 2 * C], BF16, tag="sqs")
                for r2 in range(NB // 4):
                    ps_cv = psC.tile([128, 2, C], F32, space="PSUM", tag="cv")
                    for h in range(2):
                        nc.tensor.matmul(
                            ps_cv[64 * h:64 * (h + 1), :, :]
                                .rearrange("p a c -> p (a c)"),
                            lhsT=acwt[:],
                            rhs=uTs[:, 4 * r2 + 2 * h:4 * r2 + 2 * h + 2, :]
                                .rearrange("p a c -> p (a c)"))
                    nc.scalar.activation(
                        sqs[:, r2, :], ps_cv[:].rearrange("p a c -> p (a c)"),
                        AF.Square, bias=acb2[:])

                # ---- num(+beta) and cn matmuls into ps_big ----
                # cols: num1 [0:128] (j*T+t), num2 [128:256] (parts 0:72),
                #       cn1 [256:272], cn2 [272:288] (parts 0:72)
                ps_big = psN.tile([C1, NW], F32, space="PSUM", tag="big")
                for j in range(NB):
                    r2, i = j // 4, j % 4
                    p0, cb = 64 * (i // 2), C * (i % 2)
                    nc.tensor.matmul(
                        ps_big[:, 256 + j:257 + j],
                        lhsT=sqs[p0:p0 + DA, r2, cb:cb + C1],
                        rhs=onesb[p0:p0 + DA, :])
                    nc.tensor.matmul(
                        ps_big[0:C2, 272 + j:273 + j],
                        lhsT=sqs[p0:p0 + DA, r2, cb + C1:cb + C],
                        rhs=onesb[p0:p0 + DA, :])
                    ts = slice(j * T, (j + 1) * T)
                    nc.tensor.matmul(ps_big[:, j * T:(j + 1) * T],
                                     lhsT=uTs[:, j, 0:C1],
                                     rhs=HTsb[:, ts], start=True, stop=False,
                                     skip_group_check=True)
                    nc.tensor.matmul(ps_big[0:C2, 128 + j * T:128 + (j + 1) * T],
                                     lhsT=uTs[:, j, C1:C],
                                     rhs=HTsb[:, ts], start=True, stop=False,
                                     skip_group_check=True)
                # batched beta accumulate (one per chunk)
                nc.tensor.matmul(ps_big[:, 0:128], lhsT=ones_row[:, 0:C1],
                                 rhs=berow[:], start=False, stop=True,
                                 skip_group_check=True)
                nc.tensor.matmul(ps_big[0:C2, 128:256], lhsT=ones_row[:, 0:C2],
                                 rhs=berow[:], start=False, stop=True,
                                 skip_group_check=True)

                # ---- invcn = rsqrt(cn^2) (batched), broadcast scale ----
                lcn1 = ap_.tile([C1, NB], F32, tag="lcn1")
                nc.scalar.activation(lcn1[:], ps_big[:, 256:272], AF.Ln)
                invcn1 = ap_.tile([C1, NB], F32, tag="invcn1")
                nc.scalar.activation(invcn1[:], lcn1[:], AF.Exp, scale=-0.5)
                lcn2 = ap_.tile([C2, NB], F32, tag="lcn2")
                nc.scalar.activation(lcn2[:], ps_big[0:C2, 272:288], AF.Ln)
                invcn2 = ap_.tile([C2, NB], F32, tag="invcn2")
                nc.scalar.activation(invcn2[:], lcn2[:], AF.Exp, scale=-0.5)
                n1v = ps_big[:, 0:128].rearrange("p (a b) -> p a b", b=T)
                nc.vector.tensor_tensor(
                    n1v, n1v,
                    invcn1[:, :, None].to_broadcast((C1, NB, T)), OP.mult)
                n2v = ps_big[0:C2, 128:256].rearrange("p (a b) -> p a b", b=T)
                nc.vector.tensor_tensor(
                    n2v, n2v,
                    invcn2[:, :, None].to_broadcast((C2, NB, T)), OP.mult)

                # ---- exp (pure, batched), multiplicative mask (DVE) ----
                attn1 = ap_.tile([C1, NB, T], BF16, tag="attn1")
                nc.scalar.activation(
                    attn1[:].rearrange("p a b -> p (a b)"),
                    ps_big[:, 0:128], AF.Exp)
                attn2 = ap_.tile([C2, NB, T], BF16, tag="attn2")
                nc.scalar.activation(
                    attn2[:].rearrange("p a b -> p (a b)"),
                    ps_big[0:C2, 128:256], AF.Exp)
                nc.vector.tensor_tensor(
                    attn1[:], attn1[:],
                    mb1[:, g * NB:(g + 1) * NB][:, :, None]
                        .to_broadcast((C1, NB, T)), OP.mult)
                nc.vector.tensor_tensor(
                    attn2[:], attn2[:],
                    mb2[:, g * NB:(g + 1) * NB][:, :, None]
                        .to_broadcast((C2, NB, T)), OP.mult)

                # ---- s = u.T @ attn ; Sigma ----
                ps_sT = psS.tile([E, NB, T], F32, space="PSUM", tag="sT")
                for j in range(NB):
                    nc.tensor.matmul(ps_sT[:, j, :], lhsT=ug1[:, gs + j, :],
                                     rhs=attn1[:, j, :], start=True, stop=False)
                    nc.tensor.matmul(ps_sT[:, j, :], lhsT=ug2[:, gs + j, :],
                                     rhs=attn2[:, j, :], start=False, stop=True)
                sTb = zp.tile([E, NB * T], BF16, tag="sTb")
                nc.vector.tensor_copy(sTb[:],
                                      ps_sT[:].rearrange("p a b -> p (a b)"))

                ps_S = psG.tile([128, 1], F32, space="PSUM", tag="g1")
                nc.tensor.matmul(ps_S[:],
                                 lhsT=attn1[:].rearrange("p a b -> p (a b)"),
                                 rhs=onesb[0:C1, :], start=True, stop=False)
                nc.tensor.matmul(ps_S[:],
                                 lhsT=attn2[:].rearrange("p a b -> p (a b)"),
                                 rhs=onesb[0:C2, :], start=False, stop=True)
                invS = zp.tile([128, 1], F32, tag="invS")
                nc.vector.reciprocal(invS[:], ps_S[:])

                # ---- z path ----
                ps_yT = psG.tile([E, 128], F32, space="PSUM", tag="g1")
                nc.tensor.matmul(ps_yT[:], lhsT=bcwt[:], rhs=sTb[:])
                yT = zp.tile([E, 128], BF16, tag="yT")
                nc.scalar.copy(yT[:], ps_yT[:])
                ps_zT = psG.tile([E, 128], F32, space="PSUM", tag="g1")
                nc.tensor.matmul(ps_zT[:], lhsT=rwt[:], rhs=yT[:])
                zT = zp.tile([E, 128], BF16, tag="zT")
                nc.scalar.copy(zT[:], ps_zT[:])
                ps_z = psG.tile([128, E], F32, space="PSUM", tag="g1")
                nc.tensor.matmul(ps_z[:], lhsT=zT[:], rhs=idb[:])
                zout = zp.tile([128, E], F32, tag="zout")
                nc.vector.scalar_tensor_tensor(
                    out=zout[:], in0=ps_z[:], scalar=invS[:], in1=c2b[:],
                    op0=OP.mult, op1=OP.add)
                nc.sync.dma_start(z_dram[g * 128:(g + 1) * 128, :], zout[:])

    nc.compile()
    return nc


def _prep_core_inputs(inputs, k):
    bf = ml_dtypes.bfloat16
    sl = slice(k * BL, (k + 1) * BL)
    tit = np.ascontiguousarray(
        inputs["batch_titems"][sl].astype(np.int32).reshape(NG, NB * T).T)
    cit = inputs["batch_citems"][sl].astype(np.int32)
    mask = np.asarray(inputs["mask_pad_ids"][sl])
    cit = np.where(mask, PAD, cit).T          # masked items -> zero PAD row
    mb = (~mask).astype(np.float32).astype(bf).T   # 1.0 valid / 0.0 masked
    acb = np.asarray(inputs["Ac_b"], dtype=np.float32).reshape(DA, 1)
    acwt_pad = np.zeros((E, 64), dtype=np.float32)
    acwt_pad[:, 0:DA] = np.asarray(inputs["Ac_w"]).T
    acb2 = np.zeros((128, 1), dtype=np.float32)
    acb2[0:DA] = acb
    acb2[64:64 + DA] = acb
    m = {
        "tvec": np.asarray(inputs["tvec_w"], dtype=np.float32),
        "cvec": np.asarray(inputs["cvec_w"], dtype=np.float32),
        "acwt": acwt_pad.astype(bf),
        "acwd": np.ascontiguousarray(inputs["Ac_w"]).astype(bf),
        "acbb": acb.astype(bf),
        "acb2": acb2,
        "atwt": np.ascontiguousarray(inputs["At_w"].T).astype(bf),
        "atb": np.asarray(inputs["At_b"], dtype=np.float32).reshape(DA, 1),
        "bcwt": np.ascontiguousarray(inputs["Bc_w"].T).astype(bf),
        "rwt": np.ascontiguousarray(inputs["R_w"].T).astype(bf),
        "rwt32": np.ascontiguousarray(inputs["R_w"].T).astype(np.float32),
        "bcb32": np.asarray(inputs["Bc_b"], dtype=np.float32).reshape(E, 1),
        "rb32": np.asarray(inputs["R_b"], dtype=np.float32).reshape(E, 1),
        "cit1": np.ascontiguousarray(cit[0:C1]),
        "cit2": np.ascontiguousarray(cit[C1:C]),
        "mb1": np.ascontiguousarray(mb[0:C1]),
        "mb2": np.ascontiguousarray(mb[C1:C]),
        "titg": tit,
    }
    return m


def _install_profile_hook():
    """Dev-only: register the axon NTFF hook missing from this image."""
    import sys
    import types
    try:
        import antenv.axon_hooks  # noqa: F401
        return
    except ImportError:
        pass
    from trn_agent_boot.trn_boot import _ntff_profile_via_ctypes
    hook = _ntff_profile_via_ctypes("/opt/axon/libaxon_pjrt.so")
    mod = types.ModuleType("antenv.axon_hooks")
    mod._hook = hook
    mod.set_axon_ntff_profile_hook = lambda h: setattr(mod, "_hook", h)
    mod.get_axon_ntff_profile_hook = lambda: mod._hook
    sys.modules["antenv.axon_hooks"] = mod
    import antenv
    antenv.axon_hooks = mod


def kernel(**inputs) -> np.ndarray:
    if "nc" not in _CACHE:
        _CACHE["nc"] = _build()
    nc = _CACHE["nc"]
    inputs = {k: np.asarray(v) for k, v in inputs.items()}
    in_maps = [_prep_core_inputs(inputs, k) for k in range(NCORES)]
    trace = bool(int(os.environ.get("KERNEL_TRACE", "0")))
    kw = {}
    if trace:
        try:
            _install_profile_hook()
            import concourse.bass_utils as _bu
            _bu.upload_artifacts = lambda d: d
            tdir = os.environ.get("KERNEL_TRACE_DIR", "/root/problem/_trace")
            os.makedirs(tdir, exist_ok=True)
            kw["tmpdir"] = tdir
        except Exception as e:  # profiling is best-effort
            print(f"trace setup failed: {e}")
            trace = False
    res = run_bass_kernel_spmd(
        nc, in_maps, list(range(NCORES)), trace=trace, **kw,
    )
    _CACHE["last_result"] = res
    z = np.concatenate(
        [res.results[k]["z_out"].reshape(BL, T, E) for k in range(NCORES)], axis=0
    )
    return z.astype(np.float32)
